# revision 4
# baseline (speedup 1.0000x reference)
"""DeltaDequantization Trainium2 kernel (8-core SPMD, pure data parallel over batch).

Math (per batch element b, chunks c of 32 steps):
    scale_c = (1/32) * sum_{s,n} x[b,c,s,n] * cs[n]          (independent of carry!)
    S_c     = prod_{c'<c} scale_c'          (exclusive cumprod)
    y[b,t]  = sum_n x[b,t,n] * qb[n]
    m_c     = (1/32) * sum_{s in c} y[b,t]
    pred_c  = sum_{c'<c} S_c' * m_c'        (exclusive cumsum)
    out[b,t]= pred_c(t) + S_c(t) * y[b,t]

Kernel: fully strip-pipelined (strip = 64 timesteps = one 1 MiB cast-load).
Per strip: SWDGE cast-load f32->bf16 [128b, 2048], 16 PE 128x128 transposes
to (t'',n)-on-partitions layout, 4 accumulating [128,32]x[128,512] matmuls
(qb and cs columns interleaved) into a rotated 32-row PSUM band, PSUM->SBUF
band copy, 4 small [32,128] PE transposes back to batch-on-partitions,
chunk reduces, 2-chunk incremental scans (cumprod/cumsum carried across
strips), affine, per-span (256 t) store.

DVE ops are restricted to tensor_tensor / tensor_reduce / scan classes
(single-port) so SWDGE descriptor generation on GpSimd is never stalled by
a DVE 2-port op; bulk PSUM->SBUF copies go on ACT (never contends).
"""

import numpy as np

import concourse.bass as bass
import concourse.bacc as bacc
import concourse.tile as tile
from concourse import mybir
from concourse.bass_utils import run_bass_kernel_spmd
from concourse.masks import make_identity

F32 = mybir.dt.float32
BF16 = mybir.dt.bfloat16

B, T, NB = 1024, 2048, 32
NCORES = 8
BS = B // NCORES          # 128 batch rows per core = full partition dim
ADAPT = 32
C = T // ADAPT            # 64 chunks
STRIP_T = 64              # timesteps per strip = one 1 MiB f32 load
NSTRIP = T // STRIP_T     # 32
SF = STRIP_T * NB         # 2048 elements per partition per strip
PREFETCH = 14             # strip loads in flight ahead of compute

_cached_nc = None


def build_kernel():
    nc = bacc.Bacc("TRN2", target_bir_lowering=False, debug=False)

    x_ext = nc.dram_tensor("x", [BS, T * NB], F32, kind="ExternalInput")
    qb_ext = nc.dram_tensor("quant_bins", [NB, 1], F32, kind="ExternalInput")
    cs_ext = nc.dram_tensor("change_scales", [NB, 1], F32, kind="ExternalInput")
    out_ext = nc.dram_tensor("out", [BS, T], F32, kind="ExternalOutput")

    ADD = mybir.AluOpType.add
    MUL = mybir.AluOpType.mult
    BYP = mybir.AluOpType.bypass

    with tile.TileContext(nc) as tc:
        with (
            tc.tile_pool(name="consts", bufs=1) as consts,
            tc.tile_pool(name="xpool", bufs=16) as xpool,
            tc.tile_pool(name="xtpool", bufs=3) as xtpool,
            tc.tile_pool(name="ywpool", bufs=2) as ywpool,
            tc.tile_pool(name="ypool", bufs=3) as ypool,
            tc.tile_pool(name="accpool", bufs=1) as accpool,
            tc.tile_pool(name="ps_t", bufs=3, space="PSUM") as ps_t,
            tc.tile_pool(name="ps_b", bufs=2, space="PSUM") as ps_b,
            tc.tile_pool(name="ps_s", bufs=2, space="PSUM") as ps_s,
        ):
            # small consts / scan chains (DVE memsets are tiny: no Q7 stall)
            zb = consts.tile([128, 1], F32)
            inv32 = consts.tile([128, 1], F32)
            nc.vector.memset(zb[:], 0.0)
            nc.vector.memset(inv32[:], 1.0 / ADAPT)
            S_chain = consts.tile([128, C + 1], F32)
            pred_chain = consts.tile([128, C + 1], F32)
            nc.vector.memset(S_chain[:, 0:1], 1.0)
            nc.vector.memset(pred_chain[:, 0:1], 0.0)
            m_buf = consts.tile([128, C], F32)
            p_buf = consts.tile([128, C], F32)
            tau_buf = consts.tile([128, C], F32)

            # qb/cs staging via HWDGE (keeps SWDGE ring free)
            qbcs = consts.tile([128, 2], F32)
            for tp in range(4):
                nc.sync.dma_start(out=qbcs[32 * tp:32 * tp + 32, 0:1], in_=qb_ext[:])
                nc.sync.dma_start(out=qbcs[32 * tp:32 * tp + 32, 1:2], in_=cs_ext[:])

            # first x loads: get SWDGE streaming as early as possible
            xh = [
                xpool.tile([128, SF], BF16, name="xh", tag="xh")
                for _ in range(NSTRIP)
            ]

            def issue_load(s):
                nc.gpsimd.dma_start(out=xh[s][:], in_=x_ext[:, s * SF:(s + 1) * SF])

            issue_load(0)
            issue_load(1)

            ident_bf = consts.tile([128, 128], BF16)
            make_identity(nc, ident_bf[:])

            for s in range(2, PREFETCH):
                issue_load(s)

            # Four stationary matrices A32_q [128, 32], q = 0..3, built on ACT.
            # Column m = 16*j + 4*q + t''; A32_q[(t', n), m] = delta(t', t'') *
            # (qb[n] if j == 0 else cs[n]/32); zero columns for other q.
            A32 = []
            for q in range(4):
                Aq = consts.tile([128, 32], BF16, tag=f"A32_{q}")
                nc.scalar.memzero(Aq[:])
                for tp in range(4):
                    sl = slice(32 * tp, 32 * tp + 32)
                    nc.scalar.mul(Aq[sl, 4 * q + tp:4 * q + tp + 1], qbcs[sl, 0:1], 1.0)
                    nc.scalar.mul(
                        Aq[sl, 16 + 4 * q + tp:16 + 4 * q + tp + 1],
                        qbcs[sl, 1:2],
                        1.0 / ADAPT,
                    )
                A32.append(Aq)

            out_sb = accpool.tile([128, T], F32)

            for s in range(NSTRIP):
                if s + PREFETCH < NSTRIP:
                    issue_load(s + PREFETCH)
                r = 32 * (s % 4)  # rotated PE column band
                x_h = xh[s]

                # 16 transposes -> (t'', n) on partitions, free = b
                xT = xtpool.tile([128, SF], BF16)
                for h in range(2):
                    pst = ps_t.tile([128, 1024], BF16)
                    for k in range(8):
                        blk = 8 * h + k
                        nc.tensor.transpose(
                            pst[:, k * 128:(k + 1) * 128],
                            x_h[:, blk * 128:(blk + 1) * 128],
                            ident_bf[:],
                        )
                    dst = xT[:, h * 1024:(h + 1) * 1024]
                    if h == 0:
                        # DVE copy as tensor_tensor (1-port; never stalls Q7)
                        nc.vector.tensor_tensor(
                            out=dst, in0=pst[:],
                            in1=zb[:, 0:1].broadcast_to([128, 1024]), op=ADD,
                        )
                    else:
                        nc.scalar.copy(out=dst, in_=pst[:])

                # y/w projection: 4 accumulating matmuls into 32-row PSUM band
                ps_band = ps_b.tile([128, 512], F32)
                for q in range(4):
                    nc.tensor.matmul(
                        ps_band[r:r + 32, :],
                        A32[q][:],
                        xT[:, q * 512:(q + 1) * 512],
                        start=(q == 0),
                        stop=(q == 3),
                        tile_position=(0, r),
                    )

                # band -> SBUF (bf16), then 4 small transposes back to b-major
                yw = ywpool.tile([128, 512], BF16)
                nc.scalar.copy(out=yw[r:r + 32, :], in_=ps_band[r:r + 32, :])
                slab = ps_s.tile([128, 128], BF16)
                for blk2 in range(4):
                    nc.tensor.transpose(
                        slab[:, 32 * blk2:32 * blk2 + 32],
                        yw[r:r + 32, 128 * blk2:128 * (blk2 + 1)],
                        ident_bf[r:r + 32, r:r + 32],
                        tile_position=(r, 0),
                    )

                # slab free index = 32*blk + 16*j + 4*q + t'';  t = 16q+4blk+t''
                ytmp = ypool.tile([128, STRIP_T], F32)
                yv = ytmp[:].rearrange("p (q blk t) -> p q blk t", q=4, blk=4, t=4)
                sv = slab[:].rearrange(
                    "p (blk j q t) -> p j q blk t", blk=4, j=2, q=4, t=4
                )
                nc.vector.tensor_tensor(
                    out=yv,
                    in0=sv[:, 0:1].squeeze(1),
                    in1=zb[:, 0:1].unsqueeze(2).unsqueeze(3).broadcast_to([128, 4, 4, 4]),
                    op=ADD,
                )
                # chunk stats: m = (1/32)*sum y ; p = sum w (cs pre-scaled)
                cs2 = slice(2 * s, 2 * s + 2)
                nc.vector.tensor_reduce(
                    out=m_buf[:, cs2],
                    in_=ytmp[:].rearrange("p (c u) -> p c u", c=2, u=ADAPT),
                    axis=mybir.AxisListType.X,
                    op=ADD,
                )
                nc.vector.tensor_tensor(
                    out=m_buf[:, cs2], in0=m_buf[:, cs2],
                    in1=inv32[:, 0:1].broadcast_to([128, 2]), op=MUL,
                )
                wv = slab[:].rearrange(
                    "p (blk j c q2 t) -> p j c q2 blk t", blk=4, j=2, c=2, q2=2, t=4
                )
                nc.vector.tensor_reduce(
                    out=p_buf[:, cs2],
                    in_=wv[:, 1:2].squeeze(1),
                    axis=mybir.AxisListType.XYZ,
                    op=ADD,
                )
                # incremental scans (2 chunks per strip)
                nc.vector.tensor_tensor_scan(
                    out=S_chain[:, 2 * s + 1:2 * s + 3],
                    data0=p_buf[:, cs2],
                    data1=p_buf[:, cs2],
                    initial=S_chain[:, 2 * s:2 * s + 1],
                    op0=MUL,
                    op1=BYP,
                )
                nc.vector.tensor_tensor(
                    out=tau_buf[:, cs2], in0=S_chain[:, cs2], in1=m_buf[:, cs2], op=MUL
                )
                nc.vector.tensor_tensor_scan(
                    out=pred_chain[:, 2 * s + 1:2 * s + 3],
                    data0=tau_buf[:, cs2],
                    data1=tau_buf[:, cs2],
                    initial=pred_chain[:, 2 * s:2 * s + 1],
                    op0=ADD,
                    op1=BYP,
                )
                # affine: out = pred_c + S_c * y
                o3 = out_sb[:, STRIP_T * s:STRIP_T * (s + 1)].rearrange(
                    "p (c u) -> p c u", c=2, u=ADAPT
                )
                y3 = ytmp[:].rearrange("p (c u) -> p c u", c=2, u=ADAPT)
                S_bc = S_chain[:, cs2].unsqueeze(2).broadcast_to([128, 2, ADAPT])
                pred_bc = pred_chain[:, cs2].unsqueeze(2).broadcast_to([128, 2, ADAPT])
                nc.vector.tensor_tensor(out=o3, in0=y3, in1=S_bc, op=MUL)
                nc.vector.tensor_tensor(out=o3, in0=o3, in1=pred_bc, op=ADD)

                if s % 4 == 3:
                    t_lo = STRIP_T * (s - 3)
                    t_hi = STRIP_T * (s + 1)
                    nc.sync.dma_start(
                        out=out_ext[:, t_lo:t_hi], in_=out_sb[:, t_lo:t_hi]
                    )

    nc.compile()
    return nc


def make_in_maps(inputs):
    x = np.ascontiguousarray(inputs["x"], dtype=np.float32)
    qb = np.ascontiguousarray(inputs["quant_bins"], dtype=np.float32).reshape(NB, 1)
    cs = np.ascontiguousarray(inputs["change_scales"], dtype=np.float32).reshape(NB, 1)
    return [
        {
            "x": x[i * BS:(i + 1) * BS].reshape(BS, T * NB),
            "quant_bins": qb,
            "change_scales": cs,
        }
        for i in range(NCORES)
    ]


def gather_out(res):
    out = np.concatenate([res.results[i]["out"] for i in range(NCORES)], axis=0)
    return out.astype(np.float32)


def kernel(x, quant_bins, change_scales):
    global _cached_nc
    if _cached_nc is None:
        _cached_nc = build_kernel()
    nc = _cached_nc

    in_maps = make_in_maps(
        {"x": x, "quant_bins": quant_bins, "change_scales": change_scales}
    )
    res = run_bass_kernel_spmd(nc, in_maps, core_ids=list(range(NCORES)))
    return gather_out(res)


if __name__ == "__main__":
    rng = np.random.default_rng(0)
    x = rng.standard_normal((B, T, NB)).astype(np.float32)
    qb = rng.standard_normal((NB,)).astype(np.float32)
    cs = rng.uniform(0.9, 1.1, (NB, 1)).astype(np.float32)
    out = kernel(x=x, quant_bins=qb, change_scales=cs)
    print("out", out.shape, out.dtype)
